# revision 1
# baseline (speedup 1.0000x reference)
"""MoE v3: routed data-parallel, matmul-based dispatch (no scatter/gather-x).

Per core (1024 tokens):
  - fp32 gate, top-2 via max8; tie-safe onehots (first-of-max + set-minus).
  - per-(tile,expert) slot ranks via triangular matmul; capacity B=48/tile.
  - dispatch: xTe[d, slot] = sum_t xb[t,d] * Sel[t,slot]  (PE matmuls; gathers
    AND transposes in one step, zero-padded slots for free).
  - per-expert fc1+relu, fc2+b2, LayerNorm -> ybuf (bf16, slot-major DRAM).
  - combine: indirect-gather each token's 2 rows, gate-weight, sum (the only
    indirect DMA in the kernel: 16 instructions).
"""

import os
import sys

import numpy as np

for _p in ("/opt/trn_rl_repo", "/root/.axon_site/_ro/trn_rl_repo"):
    if os.path.isdir(_p) and _p not in sys.path:
        sys.path.insert(0, _p)

import ml_dtypes  # noqa: E402

BF16 = ml_dtypes.bfloat16

B, S, D, H, E = 4, 2048, 512, 512, 8
T = B * S
N_CORES = 8
TC = T // N_CORES
P = 128
DC = D // P
HC = H // P
EPS = 1e-5
NTT = TC // P          # 8 token tiles
BCAP = 48              # slots per (tile, expert); real max is 46
C = NTT * BCAP         # 384 slots per expert
NS = E * C             # 3072
NSB = NS // P          # 24


def _build_nc(apply_gamma_beta: bool):
    import concourse.bass as bass
    import concourse.tile as tile
    from concourse import bacc, mybir

    f32 = mybir.dt.float32
    bf16 = mybir.dt.bfloat16
    i32 = mybir.dt.int32
    AF = mybir.ActivationFunctionType
    OP = mybir.AluOpType

    nc = bacc.Bacc()

    xT_d = nc.dram_tensor("xT", [P, DC, TC], f32, kind="ExternalInput")
    xbp_d = nc.dram_tensor("xbp", [P, NTT, D], bf16, kind="ExternalInput")
    wg_d = nc.dram_tensor("wg", [P, DC, E], f32, kind="ExternalInput")
    tri_d = nc.dram_tensor("tri", [P, P], bf16, kind="ExternalInput")
    w1_d = nc.dram_tensor("w1", [P, E, DC, H], bf16, kind="ExternalInput")
    w2_d = nc.dram_tensor("w2", [P, E, HC, D], bf16, kind="ExternalInput")
    b1_d = nc.dram_tensor("b1", [P, E, HC], f32, kind="ExternalInput")
    b2_d = nc.dram_tensor("b2", [1, E, D], bf16, kind="ExternalInput")
    if apply_gamma_beta:
        gam_d = nc.dram_tensor("gamma", [1, E, D], f32, kind="ExternalInput")
        bet_d = nc.dram_tensor("beta", [1, E, D], f32, kind="ExternalInput")
    out_d = nc.dram_tensor("out", [TC, D], f32, kind="ExternalOutput")

    ybuf_d = nc.dram_tensor("ybuf", [NS, D], bf16)

    with tile.TileContext(nc) as tc:
        with (
            tc.tile_pool(name="consts", bufs=1) as consts,
            tc.tile_pool(name="hpool", bufs=3) as hpool,
            tc.tile_pool(name="xe", bufs=3) as xepool,
            tc.tile_pool(name="ypool", bufs=3) as ypool,
            tc.tile_pool(name="small", bufs=4) as small,
            tc.tile_pool(name="pd", bufs=2, space="PSUM") as psum_d,
            tc.tile_pool(name="ph", bufs=2, space="PSUM") as psum_h,
            tc.tile_pool(name="py", bufs=2, space="PSUM") as psum_y,
            tc.tile_pool(name="pg", bufs=2, space="PSUM") as psum_g,
        ):
            # ---- small/early inputs first so the gate starts ASAP ----
            wg_sb = consts.tile([P, DC, E], f32)
            nc.sync.dma_start(out=wg_sb, in_=wg_d[:])
            xT_sb = consts.tile([P, DC, TC], f32)
            for tt in range(NTT):
                nc.sync.dma_start(
                    out=xT_sb[:, :, tt * P:(tt + 1) * P],
                    in_=xT_d[:, :, tt * P:(tt + 1) * P],
                )
            xbp_sb = consts.tile([P, NTT, D], bf16)
            nc.sync.dma_start(out=xbp_sb, in_=xbp_d[:])
            tri_sb = consts.tile([P, P], bf16)
            nc.sync.dma_start(out=tri_sb, in_=tri_d[:])
            b1_sb = consts.tile([P, E, HC], f32)
            nc.sync.dma_start(out=b1_sb, in_=b1_d[:])
            b2_sb = consts.tile([1, E, D], bf16)
            nc.sync.dma_start(out=b2_sb, in_=b2_d[:])
            if apply_gamma_beta:
                gam_sb = consts.tile([1, E, D], f32)
                nc.sync.dma_start(out=gam_sb, in_=gam_d[:])
                bet_sb = consts.tile([1, E, D], f32)
                nc.sync.dma_start(out=bet_sb, in_=bet_d[:])
            # per-expert weight loads so fc1(e) only waits on its slice
            w1_sb = consts.tile([P, E, DC, H], bf16)
            w2_sb = consts.tile([P, E, HC, D], bf16)
            for e in range(E):
                nc.sync.dma_start(out=w1_sb[:, e], in_=w1_d[:, e])
                nc.sync.dma_start(out=w2_sb[:, e], in_=w2_d[:, e])

            onesb_sb = consts.tile([1, P], bf16)
            nc.vector.memset(onesb_sb, 1.0)
            eps_sb = consts.tile([P, 1], f32)
            nc.vector.memset(eps_sb, EPS)
            rcol_sb = consts.tile([P, BCAP], f32)     # [p, r] = r
            nc.gpsimd.iota(rcol_sb, pattern=[[1, BCAP]], base=0, channel_multiplier=0, allow_small_or_imprecise_dtypes=True)
            te_sb = consts.tile([P, NTT, E], f32)     # 48*tt + 384*e
            nc.gpsimd.iota(te_sb, pattern=[[BCAP, NTT], [C, E]], base=0, channel_multiplier=0, allow_small_or_imprecise_dtypes=True)
            tt48_sb = consts.tile([P, NTT], f32)      # 48*tt
            nc.gpsimd.iota(tt48_sb, pattern=[[BCAP, NTT]], base=0, channel_multiplier=0, allow_small_or_imprecise_dtypes=True)
            pad8 = consts.tile([P, NTT, 16], f32)     # zero-padded shift scratch
            nc.vector.memset(pad8, 0.0)
            pad4 = consts.tile([P, NTT, 16], f32)
            nc.vector.memset(pad4, 0.0)
            pad2 = consts.tile([P, NTT, 16], f32)
            nc.vector.memset(pad2, 0.0)

            g01_sb = consts.tile([P, NTT, 2], f32)
            slot_sb = consts.tile([P, NTT, 2], i32)
            mask_sb = consts.tile([P, NTT, E], bf16)
            sel_sb = consts.tile([P, NTT, E, BCAP], bf16)

            # ---------- gate + slot ranks (DVE ops batched across tiles) ----
            lg_all = consts.tile([P, NTT, E], f32)
            m12_all = consts.tile([P, NTT, 2], f32)
            ex_all = consts.tile([P, NTT, E], f32)
            pos_all = consts.tile([P, NTT, E], f32)
            for tt in range(NTT):
                pg = psum_g.tile([P, E], f32, tag="pg8")
                for dc in range(DC):
                    nc.tensor.matmul(
                        out=pg,
                        lhsT=xT_sb[:, dc, tt * P:(tt + 1) * P],
                        rhs=wg_sb[:, dc, :],
                        start=(dc == 0),
                        stop=(dc == DC - 1),
                    )
                nc.vector.tensor_copy(lg_all[:, tt, :], pg)
                mx = small.tile([P, 8], f32)
                nc.vector.max(mx, lg_all[:, tt, :])
                nc.vector.tensor_copy(m12_all[:, tt, :], mx[:, 0:2])

            negm1 = small.tile([P, NTT], f32)
            nc.vector.tensor_scalar_mul(negm1, m12_all[:, :, 0], -1.0)
            for tt in range(NTT):
                nc.scalar.activation(
                    ex_all[:, tt, :], lg_all[:, tt, :], AF.Exp,
                    bias=negm1[:, tt:tt + 1], scale=1.0,
                )
            m1b = m12_all[:, :, 0:1].to_broadcast([P, NTT, E])
            m2b = m12_all[:, :, 1:2].to_broadcast([P, NTT, E])
            ge_all = consts.tile([P, NTT, E], f32)
            nc.vector.tensor_tensor(ge_all, lg_all, m2b, op=OP.is_ge)
            nc.vector.tensor_copy(mask_sb[:, :, :], ge_all)
            gts = consts.tile([P, NTT, E], f32)
            nc.vector.tensor_mul(gts, ex_all, ge_all)
            den = small.tile([P, NTT], f32)
            nc.vector.reduce_sum(den, gts, axis=mybir.AxisListType.X)
            rden = small.tile([P, NTT, 1], f32)
            nc.vector.reciprocal(rden[:, :, 0], den)
            gw_all = consts.tile([P, NTT, E], f32)
            nc.vector.tensor_tensor(
                gw_all, gts, rden.to_broadcast([P, NTT, E]), op=OP.mult
            )
            # intra-tile ranks
            for tt in range(NTT):
                pp = psum_g.tile([P, E], f32, tag="pg8")
                nc.tensor.matmul(
                    out=pp, lhsT=tri_sb[:, :], rhs=mask_sb[:, tt, :],
                    start=True, stop=True,
                )
                nc.vector.tensor_copy(pos_all[:, tt, :], pp)
            slocal = consts.tile([P, NTT, E], f32)
            nc.vector.tensor_mul(slocal, pos_all, ge_all)
            nc.vector.tensor_scalar_sub(slocal, slocal, 1.0)
            # Sel straight from ranks: col (tt,e,r) set iff slocal[t,tt,e] == r
            nc.vector.tensor_tensor(
                sel_sb,
                rcol_sb[:, None, None, :].to_broadcast([P, NTT, E, BCAP]),
                slocal[:, :, :, None].to_broadcast([P, NTT, E, BCAP]),
                op=OP.is_equal,
            )
            # tie-safe onehots
            ohm = consts.tile([P, NTT, E], f32)
            nc.vector.tensor_tensor(ohm, lg_all, m1b, op=OP.is_equal)
            nc.vector.tensor_copy(pad8[:, :, 8:16], ohm)
            s1 = consts.tile([P, NTT, E], f32)
            nc.vector.tensor_add(s1, pad8[:, :, 8:16], pad8[:, :, 7:15])
            nc.vector.tensor_copy(pad4[:, :, 8:16], s1)
            s2 = consts.tile([P, NTT, E], f32)
            nc.vector.tensor_add(s2, pad4[:, :, 8:16], pad4[:, :, 6:14])
            nc.vector.tensor_copy(pad2[:, :, 8:16], s2)
            pre = consts.tile([P, NTT, E], f32)
            nc.vector.tensor_add(pre, pad2[:, :, 8:16], pad2[:, :, 4:12])
            isone = consts.tile([P, NTT, E], f32)
            nc.vector.tensor_scalar(isone, pre, 1.0, None, op0=OP.is_equal)
            oh0 = consts.tile([P, NTT, E], f32)
            nc.vector.tensor_mul(oh0, ohm, isone)
            oh1 = consts.tile([P, NTT, E], f32)
            nc.vector.tensor_sub(oh1, ge_all, oh0)
            tk = consts.tile([P, NTT, E], f32)
            nc.vector.tensor_mul(tk, oh0, gw_all)
            nc.vector.reduce_sum(g01_sb[:, :, 0], tk, axis=mybir.AxisListType.X)
            tk1 = consts.tile([P, NTT, E], f32)
            nc.vector.tensor_mul(tk1, oh1, gw_all)
            nc.vector.reduce_sum(g01_sb[:, :, 1], tk1, axis=mybir.AxisListType.X)

            sg = consts.tile([P, NTT, E], f32)
            nc.vector.tensor_add(sg, slocal, te_sb)
            skr_all = consts.tile([P, 2, NTT], f32)
            for k, oh in ((0, oh0), (1, oh1)):
                sk = consts.tile([P, NTT, E], f32, tag=f"sk{k}")
                nc.vector.tensor_mul(sk, oh, sg)
                nc.vector.reduce_sum(
                    skr_all[:, k, :], sk, axis=mybir.AxisListType.X
                )
                nc.vector.tensor_copy(slot_sb[:, :, k], skr_all[:, k, :])
            # ---------- dispatch + experts (per-expert, PE-dense) ----------
            def emit_fc1(e):
                xg = xepool.tile([P, DC, C], bf16, tag="xg")
                for dc in range(DC):
                    pse = psum_d.tile([P, C], f32)
                    for tt in range(NTT):
                        nc.tensor.matmul(
                            out=pse[:, tt * BCAP:(tt + 1) * BCAP],
                            lhsT=xbp_sb[:, tt, dc * P:(dc + 1) * P],
                            rhs=sel_sb[:, tt, e, :],
                            start=True, stop=True,
                        )
                    nc.scalar.copy(out=xg[:, dc, :], in_=pse)
                hT = hpool.tile([P, HC, C], bf16, tag="hT")
                hts[e] = hT
                for hc in range(HC):
                    ph = psum_h.tile([P, C], f32)
                    for dc in range(DC):
                        nc.tensor.matmul(
                            out=ph,
                            lhsT=w1_sb[:, e, dc, hc * P:(hc + 1) * P],
                            rhs=xg[:, dc, :],
                            start=(dc == 0),
                            stop=(dc == DC - 1),
                        )
                    nc.scalar.activation(
                        hT[:, hc, :], ph, AF.Relu,
                        bias=b1_sb[:, e, hc:hc + 1], scale=1.0,
                    )

            hts = {}

            def emit_fc2_ln(e):
                hT = hts.pop(e)
                for ts in range(C // P):
                    py = psum_y.tile([P, D], f32)
                    nc.tensor.matmul(
                        out=py, lhsT=onesb_sb[0:1, :], rhs=b2_sb[0:1, e, :],
                        start=True, stop=False,
                    )
                    for hc in range(HC):
                        nc.tensor.matmul(
                            out=py,
                            lhsT=hT[:, hc, ts * P:(ts + 1) * P],
                            rhs=w2_sb[:, e, hc, :],
                            start=False,
                            stop=(hc == HC - 1),
                        )
                    stats = small.tile([P, 6], f32)
                    nc.vector.bn_stats(stats, py)
                    mv = small.tile([P, 2], f32)
                    nc.vector.bn_aggr(mv, stats)
                    sd = small.tile([P, 1], f32)
                    nc.scalar.activation(
                        sd, mv[:, 1:2], AF.Sqrt, bias=eps_sb[:, 0:1], scale=1.0
                    )
                    rstd = small.tile([P, 1], f32)
                    nc.vector.reciprocal(rstd, sd)
                    bb = small.tile([P, 1], f32)
                    nc.vector.tensor_mul(bb, mv[:, 0:1], rstd)
                    nc.vector.tensor_scalar_mul(bb, bb, -1.0)
                    yt = ypool.tile([P, D], bf16, tag="yt")
                    nc.scalar.activation(
                        yt, py, AF.Identity, bias=bb[:, 0:1], scale=rstd[:, 0:1]
                    )
                    if apply_gamma_beta:
                        ytf = ypool.tile([P, D], f32, tag="ytf")
                        nc.vector.tensor_mul(
                            ytf, yt, gam_sb[0:1, e, :].to_broadcast([P, D])
                        )
                        nc.vector.tensor_add(
                            ytf, ytf, bet_sb[0:1, e, :].to_broadcast([P, D])
                        )
                        nc.vector.tensor_copy(yt, ytf)
                    nc.sync.dma_start(
                        out=ybuf_d[e * C + ts * P: e * C + (ts + 1) * P, :], in_=yt
                    )

            for e in range(E):
                emit_fc1(e)
                emit_fc2_ln(e)

            # ---------- combine ----------
            for tt in range(NTT):
                y0 = ypool.tile([P, D], bf16, tag="y0")
                nc.gpsimd.indirect_dma_start(
                    out=y0[:], out_offset=None, in_=ybuf_d[:],
                    in_offset=bass.IndirectOffsetOnAxis(
                        ap=slot_sb[:, tt, 0:1], axis=0
                    ),
                )
                y1 = ypool.tile([P, D], bf16, tag="y1")
                nc.gpsimd.indirect_dma_start(
                    out=y1[:], out_offset=None, in_=ybuf_d[:],
                    in_offset=bass.IndirectOffsetOnAxis(
                        ap=slot_sb[:, tt, 1:2], axis=0
                    ),
                )
                o0 = ypool.tile([P, D], f32, tag="o0")
                nc.vector.tensor_scalar_mul(o0, y0, g01_sb[:, tt, 0:1])
                o1 = ypool.tile([P, D], f32, tag="o1")
                nc.vector.tensor_scalar_mul(o1, y1, g01_sb[:, tt, 1:2])
                nc.vector.tensor_add(o0, o0, o1)
                nc.sync.dma_start(out=out_d[tt * P:(tt + 1) * P, :], in_=o0)

    nc.compile()
    return nc


def _prep_in_maps(x, Wg, W1, b1, W2, b2, gamma, beta, apply_gamma_beta):
    xf = np.ascontiguousarray(x.reshape(T, D))
    w1b = np.ascontiguousarray(
        np.transpose(W1.astype(BF16).reshape(E, DC, P, H), (2, 0, 1, 3))
    )
    w2b = np.ascontiguousarray(
        np.transpose(W2.astype(BF16).reshape(E, HC, P, D), (2, 0, 1, 3))
    )
    wgp = np.ascontiguousarray(np.transpose(Wg.reshape(DC, P, E), (1, 0, 2)))
    b1p = np.ascontiguousarray(np.transpose(b1.reshape(E, HC, P), (2, 0, 1)))
    b2p = np.ascontiguousarray(b2.astype(BF16).reshape(1, E, D))
    tri = np.tril(np.ones((P, P), np.float32)).T.astype(BF16)

    in_maps = []
    for c in range(N_CORES):
        shard = xf[c * TC:(c + 1) * TC]
        xT = np.ascontiguousarray(shard.T)
        xTp = np.ascontiguousarray(np.transpose(xT.reshape(DC, P, TC), (1, 0, 2)))
        xbp = np.ascontiguousarray(
            np.transpose(shard.astype(BF16).reshape(NTT, P, D), (1, 0, 2))
        )
        m = {
            "xT": xTp,
            "xbp": xbp,
            "w1": w1b,
            "w2": w2b,
            "wg": wgp,
            "b1": b1p,
            "b2": b2p,
            "tri": tri,
        }
        if apply_gamma_beta:
            m["gamma"] = np.ascontiguousarray(gamma.reshape(1, E, D))
            m["beta"] = np.ascontiguousarray(beta.reshape(1, E, D))
        in_maps.append(m)
    return in_maps


def run(inputs, trace=False):
    from concourse.bass_utils import run_bass_kernel_spmd

    x = np.asarray(inputs["x"], np.float32)
    Wg = np.asarray(inputs["Wg"], np.float32)
    W1 = np.asarray(inputs["W1"], np.float32)
    b1 = np.asarray(inputs["b1"], np.float32)
    W2 = np.asarray(inputs["W2"], np.float32)
    b2 = np.asarray(inputs["b2"], np.float32)
    gamma = np.asarray(inputs["gamma"], np.float32)
    beta = np.asarray(inputs["beta"], np.float32)

    apply_gb = not (np.all(gamma == 1.0) and np.all(beta == 0.0))
    nc = _build_nc(apply_gb)
    in_maps = _prep_in_maps(x, Wg, W1, b1, W2, b2, gamma, beta, apply_gb)
    res = run_bass_kernel_spmd(nc, in_maps, list(range(N_CORES)), trace=trace)
    out = np.concatenate(
        [np.asarray(res.results[c]["out"], np.float32) for c in range(N_CORES)],
        axis=0,
    )
    return out.reshape(B, S, D), res


def kernel(**inputs) -> np.ndarray:
    out, _ = run(inputs, trace=False)
    return out

